# revision 3
# baseline (speedup 1.0000x reference)
"""DeepSeek-V3 style MoE gate (nn_Gate) for Trainium2, 8-core data-parallel.

Contract: kernel(**inputs) takes the FULL inputs
    x [8192, 7168] f32, token_mask [8192] bool (unused by the reference),
    weight [256, 7168] f32, bias [256] f32
and returns (weights [8192, 8] f32, idx [8192, 8] int32), matching
reference() semantics.

Strategy:
- Token dim sharded across 8 NeuronCores (1024 tokens/core); router weight
  and bias replicated (data-parallel, per the sharding hint).
- Logits via a 1.5-pass decomposition per 128-K chunk:
    MM1 (fp16):      t(x*8) . t(w*2^14)            -> 2^17 * t(x)t(w)
    MM2 (DoubleRow): (w*2^6, wr*2^17).(xr*2^11, x) -> 2^17 * (w*xr + wr*x)
  where t() is the fp16 rounding, xr = x - t(x), wr = w - t(w). Both MMs
  accumulate into the SAME PSUM bank (identical 2^17 scale), so
  sigmoid(ps * 2^-17) gives the scores with ~2^-15 relative logit error.
- The e4m3 plane of x itself (slot1 moving) is cast on-chip from the fp16
  tile (ACT), so the x stream is 3 bytes/element (fp16 + one fp8 plane).
- PE transposes 128x128 blocks to token-major; DVE runs the group-limited
  top-8 selection (max8 / max_index / match_replace).
"""
import numpy as np
import ml_dtypes
import concourse.bass as bass
import concourse.tile as tile
from concourse import bacc, mybir
from concourse.masks import make_identity
from concourse.bass_utils import run_bass_kernel_spmd

AOT = mybir.AluOpType
F32 = mybir.dt.float32
F16 = mybir.dt.float16
E4 = mybir.dt.float8e4

N_TOKENS = 8192
K = 7168
NK = K // 128
E = 256
N_CORES = 8
T_CORE = N_TOKENS // N_CORES
TB = 512
NTB = T_CORE // TB  # 2


def _topk_chain(nc, pool, scores, bias_b, wout, iout):
    """Group-limited top-8 for one 128-token tile.

    scores: [128,256] f32 SBUF sigmoid scores; bias_b: [128,256] f32
    broadcast bias; wout/iout: DRAM APs [128,8] f32/int32.
    """
    u = pool.tile([128, 256], F32, tag="u")
    nc.vector.tensor_add(u[:], scores[:], bias_b[:])
    u3 = u[:].rearrange("p (g e) -> p g e", g=8)
    # top-2 per group of 32: group max, zap it, group max again
    gmax1 = pool.tile([128, 8], F32, tag="gmax1")
    nc.vector.tensor_reduce(gmax1[:], u3, axis=mybir.AxisListType.X, op=AOT.max)
    u_z = pool.tile([128, 256], F32, tag="u_z")
    nc.vector.match_replace(u_z[:], gmax1[:], u[:], -1e30)
    gmax2 = pool.tile([128, 8], F32, tag="gmax2")
    nc.vector.tensor_reduce(gmax2[:], u_z[:].rearrange("p (g e) -> p g e", g=8),
                            axis=mybir.AxisListType.X, op=AOT.max)
    g2sum = pool.tile([128, 8], F32, tag="g2sum")
    nc.vector.tensor_add(g2sum[:], gmax1[:], gmax2[:])
    # top-4 groups: threshold at 4th largest group score
    gtop = pool.tile([128, 8], F32, tag="gtop")
    nc.vector.max(out=gtop[:], in_=g2sum[:])
    gmask = pool.tile([128, 8], F32, tag="gmask")
    nc.vector.tensor_scalar(gmask[:], g2sum[:], gtop[:, 3:4], None, op0=AOT.is_ge)
    # mask = multiply by 0/1 exactly like the reference
    u_m = pool.tile([128, 256], F32, tag="u_m")
    nc.vector.tensor_tensor(
        out=u_m[:].rearrange("p (g e) -> p g e", g=8),
        in0=u3,
        in1=gmask[:].unsqueeze(-1).to_broadcast([128, 8, 32]),
        op=AOT.mult,
    )
    # global top-8 of masked biased scores
    fvals = pool.tile([128, 8], F32, tag="fvals")
    nc.vector.max(out=fvals[:], in_=u_m[:])
    fidx = pool.tile([128, 8], mybir.dt.uint32, tag="fidx")
    nc.vector.max_index(fidx[:], fvals[:], u_m[:])
    # mark selected positions, pull original sigmoid scores there
    u2 = pool.tile([128, 256], F32, tag="u2")
    nc.vector.match_replace(u2[:], fvals[:], u_m[:], 1e38)
    sel01 = pool.tile([128, 256], F32, tag="sel01")
    nc.vector.tensor_scalar(sel01[:], u2[:], 1e30, None, op0=AOT.is_ge)
    wsel = pool.tile([128, 256], F32, tag="wsel")
    nc.vector.tensor_mul(wsel[:], scores[:], sel01[:])
    wvals = pool.tile([128, 8], F32, tag="wvals")
    nc.vector.max(out=wvals[:], in_=wsel[:])
    widx = pool.tile([128, 8], mybir.dt.uint32, tag="widx")
    nc.vector.max_index(widx[:], wvals[:], wsel[:])
    # align score-ordered (wvals, widx) pairs to the biased order fidx
    fidx_f = pool.tile([128, 8], F32, tag="fidx_f")
    nc.vector.tensor_copy(fidx_f[:], fidx[:])
    widx_f = pool.tile([128, 8], F32, tag="widx_f")
    nc.vector.tensor_copy(widx_f[:], widx[:])
    eq = pool.tile([128, 64], F32, tag="eq")
    nc.vector.tensor_tensor(
        out=eq[:].rearrange("p (a b) -> p a b", a=8),
        in0=fidx_f[:].unsqueeze(-1).to_broadcast([128, 8, 8]),
        in1=widx_f[:].unsqueeze(1).to_broadcast([128, 8, 8]),
        op=AOT.is_equal,
    )
    wa = pool.tile([128, 64], F32, tag="wa")
    nc.vector.tensor_tensor(
        out=wa[:].rearrange("p (a b) -> p a b", a=8),
        in0=eq[:].rearrange("p (a b) -> p a b", a=8),
        in1=wvals[:].unsqueeze(1).to_broadcast([128, 8, 8]),
        op=AOT.mult,
    )
    w_al = pool.tile([128, 8], F32, tag="w_al")
    nc.vector.tensor_reduce(w_al[:], wa[:].rearrange("p (a b) -> p a b", a=8),
                            axis=mybir.AxisListType.X, op=AOT.add)
    # renormalize and scale by 2.5
    denom = pool.tile([128, 1], F32, tag="denom")
    nc.vector.tensor_reduce(denom[:], w_al[:], axis=mybir.AxisListType.X, op=AOT.add)
    recip = pool.tile([128, 1], F32, tag="recip")
    nc.vector.reciprocal(recip[:], denom[:])
    wfin = pool.tile([128, 8], F32, tag="wfin")
    nc.vector.tensor_scalar(wfin[:], w_al[:], recip[:, 0:1], 2.5, op0=AOT.mult, op1=AOT.mult)
    nc.gpsimd.dma_start(wout, wfin[:])
    nc.gpsimd.dma_start(iout, fidx[:].bitcast(mybir.dt.int32))


def build_kernel(reps=None):
    nc = bacc.Bacc("TRN2", target_bir_lowering=False, debug=False,
                   enable_asserts=False, num_devices=N_CORES)
    x16_in = nc.dram_tensor("x16", [K, T_CORE], F16, kind="ExternalInput").ap()
    xr8_in = nc.dram_tensor("xr8", [K, T_CORE], E4, kind="ExternalInput").ap()
    w16_in = nc.dram_tensor("w16", [K, E], F16, kind="ExternalInput").ap()
    wp_in = nc.dram_tensor("wp", [K, 2, E], E4, kind="ExternalInput").ap()
    bias_in = nc.dram_tensor("biasb", [128, E], F32, kind="ExternalInput").ap()
    wout = nc.dram_tensor("wout", [T_CORE, 8], F32, kind="ExternalOutput").ap()
    iout = nc.dram_tensor("iout", [T_CORE, 8], mybir.dt.int32, kind="ExternalOutput").ap()

    import contextlib
    with tile.TileContext(nc) as tc:
        with (
            tc.tile_pool(name="wres", bufs=1) as wres,
            tc.tile_pool(name="consts", bufs=1) as consts,
            tc.tile_pool(name="xs", bufs=10) as xs,
            tc.tile_pool(name="xps", bufs=10) as xps,
            tc.tile_pool(name="mmps", bufs=1, space="PSUM") as mmps,
            tc.tile_pool(name="tps", bufs=2, space="PSUM") as tps,
            tc.tile_pool(name="sig", bufs=4) as sigp,
            tc.tile_pool(name="sc", bufs=3) as scp,
            tc.tile_pool(name="chain", bufs=2) as chain,
        ):
            w16s = wres.tile([128, NK * E], F16, tag="w16s")
            nc.sync.dma_start(
                w16s[:].rearrange("p (nk e) -> p nk e", e=E),
                w16_in.rearrange("(nk p) e -> p nk e", p=128),
            )
            wps = wres.tile([128, NK * 2 * E], E4, tag="wps")
            nc.sync.dma_start(
                wps[:].rearrange("p (nk two e) -> p nk two e", e=E, two=2),
                wp_in.rearrange("(nk p) two e -> p nk two e", p=128),
            )
            bias_b = consts.tile([128, E], F32, tag="bias_b")
            nc.sync.dma_start(bias_b[:], bias_in[:])
            ident = consts.tile([128, 128], F32, tag="ident")
            make_identity(nc, ident[:])

            w16v = w16s[:].rearrange("p (nk e) -> p nk e", e=E)
            wpv = wps[:].rearrange("p (nk two e) -> p nk two e", e=E, two=2)

            loop_ctx = tc.For_i(0, reps, 1) if reps else contextlib.nullcontext()
            with loop_ctx:
                ps = [[mmps.tile([128, TB], F32, tag=f"ps_{eh}_{tb}",
                                 name=f"ps_{eh}_{tb}")
                       for tb in range(NTB)] for eh in range(2)]
                for k in range(NK):
                    xts, xpts = [], []
                    for tb in range(NTB):
                        xt = xs.tile([128, TB], F16, tag=f"xt{tb}")
                        nc.sync.dma_start(
                            xt[:], x16_in[k * 128:(k + 1) * 128,
                                          tb * TB:(tb + 1) * TB])
                        xpt = xps.tile([128, 2 * TB], E4, tag=f"xpt{tb}")
                        nc.sync.dma_start(
                            xpt[:, 0:TB], xr8_in[k * 128:(k + 1) * 128,
                                                 tb * TB:(tb + 1) * TB])
                        # slot1 moving plane: e4m3(x) cast from the fp16 tile
                        nc.scalar.activation(
                            xpt[:, TB:2 * TB], xt[:],
                            mybir.ActivationFunctionType.Copy, scale=0.125)
                        xts.append(xt)
                        xpts.append(xpt)
                    for eh in range(2):
                        w16c = w16v[:, k, eh * 128:(eh + 1) * 128]
                        wpc = wpv[:, k, :, eh * 128:(eh + 1) * 128]
                        for tb in range(NTB):
                            nc.tensor.matmul(
                                ps[eh][tb][:], w16c, xts[tb][:],
                                start=(k == 0), stop=False)
                        for tb in range(NTB):
                            nc.tensor.matmul(
                                ps[eh][tb][:], wpc,
                                xpts[tb][:].rearrange("p (two t) -> p two t", two=2),
                                start=False, stop=(k == NK - 1),
                                perf_mode=mybir.MatmulPerfMode.DoubleRow)

                for tb in range(NTB):
                    sig = [sigp.tile([128, TB], F32, tag=f"sig_{eh}",
                                     name=f"sig_{eh}_{tb}") for eh in range(2)]
                    for eh in range(2):
                        nc.scalar.activation(
                            sig[eh][:], ps[eh][tb][:],
                            mybir.ActivationFunctionType.Sigmoid,
                            scale=1.0 / 131072.0)
                    for col in range(TB // 128):
                        tt = tb * (TB // 128) + col
                        scores = scp.tile([128, E], F32, tag="scores")
                        for eh in range(2):
                            tp = tps.tile([128, 128], F32, tag="tp")
                            nc.tensor.transpose(
                                tp[:], sig[eh][:, col * 128:(col + 1) * 128],
                                ident[:])
                            nc.scalar.copy(scores[:, eh * 128:(eh + 1) * 128], tp[:])
                        _topk_chain(nc, chain, scores, bias_b,
                                    wout[tt * 128:(tt + 1) * 128, :],
                                    iout[tt * 128:(tt + 1) * 128, :])
    nc.compile()
    return nc


def host_prep(x, weight, bias):
    x = np.ascontiguousarray(np.asarray(x, dtype=np.float32))
    weight = np.ascontiguousarray(np.asarray(weight, dtype=np.float32))
    bias = np.asarray(bias, dtype=np.float32)
    e4 = ml_dtypes.float8_e4m3

    x16_all = (x * 8.0).astype(np.float16)                 # [N, K]
    xr_all = x - x16_all.astype(np.float32) * 0.125
    xr8_all = (xr_all * 2048.0).astype(e4)                 # [N, K]

    wT = weight.T                                          # [K, E]
    w16T = (wT * 16384.0).astype(np.float16)
    wr = wT - w16T.astype(np.float32) / 16384.0
    wpT = np.stack([(wT * 64.0), (wr * 131072.0)], axis=1).astype(e4)  # [K,2,E]
    w16T = np.ascontiguousarray(w16T)
    wpT = np.ascontiguousarray(wpT)
    biasb = np.ascontiguousarray(np.broadcast_to(bias, (128, E)))

    in_maps = []
    for c in range(N_CORES):
        sl = slice(c * T_CORE, (c + 1) * T_CORE)
        in_maps.append({
            "x16": np.ascontiguousarray(x16_all[sl].T),
            "xr8": np.ascontiguousarray(xr8_all[sl].T),
            "w16": w16T,
            "wp": wpT,
            "biasb": biasb,
        })
    return in_maps


_CACHED = {}


def kernel(x, token_mask, weight, bias):
    in_maps = host_prep(x, weight, bias)
    if "nc" not in _CACHED:
        _CACHED["nc"] = build_kernel()
    nc = _CACHED["nc"]
    res = run_bass_kernel_spmd(nc, in_maps, core_ids=list(range(N_CORES)))
    weights_full = np.concatenate([r["wout"] for r in res.results], axis=0)
    idx_full = np.concatenate([r["iout"] for r in res.results], axis=0)
    return weights_full.astype(np.float32), idx_full.astype(np.int32)


# revision 7
# speedup vs baseline: 1.1369x; 1.1369x over previous
"""DeepSeek-V3 style MoE gate (nn_Gate) for Trainium2, 8-core data-parallel.

Contract: kernel(**inputs) takes the FULL inputs
    x [8192, 7168] f32, token_mask [8192] bool (unused by the reference),
    weight [256, 7168] f32, bias [256] f32
and returns (weights [8192, 8] f32, idx [8192, 8] int32), matching
reference() semantics.

Strategy:
- Token dim sharded across 8 NeuronCores (1024 tokens/core); router weight
  and bias replicated (data-parallel, per the sharding hint).
- Logits via a 1.5-pass decomposition per 128-K chunk:
    MM1 (fp16):      t(x*8) . t(w*2^14)            -> 2^17 * t(x)t(w)
    MM2 (DoubleRow): (w*2^6, wr*2^17).(xr*2^11, x) -> 2^17 * (w*xr + wr*x)
  where t() is the fp16 rounding, xr = x - t(x), wr = w - t(w). Both MMs
  accumulate into the SAME PSUM bank (identical 2^17 scale), so
  sigmoid(ps * 2^-17) gives the scores with ~2^-15 relative logit error.
- The e4m3 plane of x itself (slot1 moving) is cast on-chip from the fp16
  tile (ACT), so the x stream is 3 bytes/element (fp16 + one fp8 plane).
- PE transposes 128x128 blocks to token-major; DVE runs the group-limited
  top-8 selection (max8 / max_index / match_replace).
"""
import numpy as np
import ml_dtypes
import concourse.bass as bass
import concourse.tile as tile
from concourse import bacc, mybir
from concourse.masks import make_identity
from concourse.bass_utils import run_bass_kernel_spmd

AOT = mybir.AluOpType
F32 = mybir.dt.float32
F16 = mybir.dt.float16
E4 = mybir.dt.float8e4

N_TOKENS = 8192
K = 7168
NK = K // 128
E = 256
N_CORES = 8
T_CORE = N_TOKENS // N_CORES
TB = 512
NTB = T_CORE // TB  # 2


def _topk_chain(nc, pool, scores, bias_b, wout, iout):
    """Group-limited top-8 for one 128-token tile.

    scores: [128,256] f32 SBUF sigmoid scores; bias_b: [128,256] f32
    broadcast bias; wout/iout: DRAM APs [128,8] f32/int32.
    """
    u = pool.tile([128, 256], F32, tag="u")
    nc.vector.tensor_add(u[:], scores[:], bias_b[:])
    u3 = u[:].rearrange("p (g e) -> p g e", g=8)
    # top-2 per group of 32: group max, zap it, group max again
    gmax1 = pool.tile([128, 8], F32, tag="gmax1")
    nc.vector.tensor_reduce(gmax1[:], u3, axis=mybir.AxisListType.X, op=AOT.max)
    u_z = pool.tile([128, 256], F32, tag="u_z")
    nc.vector.match_replace(u_z[:], gmax1[:], u[:], -1e30)
    gmax2 = pool.tile([128, 8], F32, tag="gmax2")
    nc.vector.tensor_reduce(gmax2[:], u_z[:].rearrange("p (g e) -> p g e", g=8),
                            axis=mybir.AxisListType.X, op=AOT.max)
    g2sum = pool.tile([128, 8], F32, tag="g2sum")
    nc.vector.tensor_add(g2sum[:], gmax1[:], gmax2[:])
    # top-4 groups: threshold at 4th largest group score
    gtop = pool.tile([128, 8], F32, tag="gtop")
    nc.vector.max(out=gtop[:], in_=g2sum[:])
    gmask = pool.tile([128, 8], F32, tag="gmask")
    nc.vector.tensor_scalar(gmask[:], g2sum[:], gtop[:, 3:4], None, op0=AOT.is_ge)
    # mask = multiply by 0/1 exactly like the reference
    u_m = pool.tile([128, 256], F32, tag="u_m")
    nc.vector.tensor_tensor(
        out=u_m[:].rearrange("p (g e) -> p g e", g=8),
        in0=u3,
        in1=gmask[:].unsqueeze(-1).to_broadcast([128, 8, 32]),
        op=AOT.mult,
    )
    # global top-8 of masked biased scores
    fvals = pool.tile([128, 8], F32, tag="fvals")
    nc.vector.max(out=fvals[:], in_=u_m[:])
    fidx = pool.tile([128, 8], mybir.dt.uint32, tag="fidx")
    nc.vector.max_index(fidx[:], fvals[:], u_m[:])
    # mark selected positions, pull original sigmoid scores there
    u2 = pool.tile([128, 256], F32, tag="u2")
    nc.vector.match_replace(u2[:], fvals[:], u_m[:], 1e38)
    sel01 = pool.tile([128, 256], F32, tag="sel01")
    nc.vector.tensor_scalar(sel01[:], u2[:], 1e30, None, op0=AOT.is_ge)
    wsel = pool.tile([128, 256], F32, tag="wsel")
    nc.vector.tensor_mul(wsel[:], scores[:], sel01[:])
    wvals = pool.tile([128, 8], F32, tag="wvals")
    nc.vector.max(out=wvals[:], in_=wsel[:])
    widx = pool.tile([128, 8], mybir.dt.uint32, tag="widx")
    nc.vector.max_index(widx[:], wvals[:], wsel[:])
    # align score-ordered (wvals, widx) pairs to the biased order fidx
    fidx_f = pool.tile([128, 8], F32, tag="fidx_f")
    nc.vector.tensor_copy(fidx_f[:], fidx[:])
    widx_f = pool.tile([128, 8], F32, tag="widx_f")
    nc.vector.tensor_copy(widx_f[:], widx[:])
    eq = pool.tile([128, 64], F32, tag="eq")
    nc.vector.tensor_tensor(
        out=eq[:].rearrange("p (a b) -> p a b", a=8),
        in0=fidx_f[:].unsqueeze(-1).to_broadcast([128, 8, 8]),
        in1=widx_f[:].unsqueeze(1).to_broadcast([128, 8, 8]),
        op=AOT.is_equal,
    )
    wa = pool.tile([128, 64], F32, tag="wa")
    nc.vector.tensor_tensor(
        out=wa[:].rearrange("p (a b) -> p a b", a=8),
        in0=eq[:].rearrange("p (a b) -> p a b", a=8),
        in1=wvals[:].unsqueeze(1).to_broadcast([128, 8, 8]),
        op=AOT.mult,
    )
    w_al = pool.tile([128, 8], F32, tag="w_al")
    nc.vector.tensor_reduce(w_al[:], wa[:].rearrange("p (a b) -> p a b", a=8),
                            axis=mybir.AxisListType.X, op=AOT.add)
    # renormalize and scale by 2.5
    denom = pool.tile([128, 1], F32, tag="denom")
    nc.vector.tensor_reduce(denom[:], w_al[:], axis=mybir.AxisListType.X, op=AOT.add)
    recip = pool.tile([128, 1], F32, tag="recip")
    nc.vector.reciprocal(recip[:], denom[:])
    wfin = pool.tile([128, 8], F32, tag="wfin")
    nc.vector.tensor_scalar(wfin[:], w_al[:], recip[:, 0:1], 2.5, op0=AOT.mult, op1=AOT.mult)
    nc.gpsimd.dma_start(wout, wfin[:])
    nc.gpsimd.dma_start(iout, fidx[:].bitcast(mybir.dt.int32))


def build_kernel(reps=None, skip_topk=False, fake_dma=False):
    nc = bacc.Bacc("TRN2", target_bir_lowering=False, debug=False,
                   enable_asserts=False, num_devices=N_CORES)
    x16_in = nc.dram_tensor("x16", [K, T_CORE], F16, kind="ExternalInput").ap()
    xr8_in = nc.dram_tensor("xr8", [K, T_CORE], E4, kind="ExternalInput").ap()
    w16_in = nc.dram_tensor("w16", [K, E], F16, kind="ExternalInput").ap()
    wp_in = nc.dram_tensor("wp", [K, 2, E], E4, kind="ExternalInput").ap()
    bias_in = nc.dram_tensor("biasb", [128, E], F32, kind="ExternalInput").ap()
    wout = nc.dram_tensor("wout", [T_CORE, 8], F32, kind="ExternalOutput").ap()
    iout = nc.dram_tensor("iout", [T_CORE, 8], mybir.dt.int32, kind="ExternalOutput").ap()

    import contextlib
    with tile.TileContext(nc) as tc:
        with (
            tc.tile_pool(name="wres", bufs=1) as wres,
            tc.tile_pool(name="consts", bufs=1) as consts,
            tc.tile_pool(name="xs", bufs=10) as xs,
            tc.tile_pool(name="xps", bufs=10) as xps,
            tc.tile_pool(name="mmps", bufs=1, space="PSUM") as mmps,
            tc.tile_pool(name="tps", bufs=4, space="PSUM") as tps,
            tc.tile_pool(name="sig", bufs=4) as sigp,
            tc.tile_pool(name="sc", bufs=8) as scp,
            tc.tile_pool(name="chain", bufs=2) as chain,
        ):
            w16s = wres.tile([128, NK * E], F16, tag="w16s")
            wps = wres.tile([128, NK * 2 * E], E4, tag="wps")
            WCH = 8
            for ci in range(NK // WCH):
                ks = slice(ci * WCH, (ci + 1) * WCH)
                nc.sync.dma_start(
                    w16s[:].rearrange("p (nk e) -> p nk e", e=E)[:, ks, :],
                    w16_in.rearrange("(nk p) e -> p nk e", p=128)[:, ks, :],
                )
                nc.sync.dma_start(
                    wps[:].rearrange("p (nk two e) -> p nk two e", e=E, two=2)[:, ks, :, :],
                    wp_in.rearrange("(nk p) two e -> p nk two e", p=128)[:, ks, :, :],
                )
            bias_b = consts.tile([128, E], F32, tag="bias_b")
            nc.sync.dma_start(bias_b[:], bias_in[:])
            ident = consts.tile([128, 128], F32, tag="ident")
            make_identity(nc, ident[:])

            w16v = w16s[:].rearrange("p (nk e) -> p nk e", e=E)
            wpv = wps[:].rearrange("p (nk two e) -> p nk two e", e=E, two=2)

            loop_ctx = tc.For_i(0, reps, 1) if reps else contextlib.nullcontext()
            with loop_ctx:
                ps = [[mmps.tile([128, TB], F32, tag=f"ps_{eh}_{tb}",
                                 name=f"ps_{eh}_{tb}")
                       for tb in range(NTB)] for eh in range(2)]
                fake_tiles = None
                for k in range(NK):
                    if fake_dma and fake_tiles is not None:
                        xts, xpts = fake_tiles
                    else:
                        xts, xpts = [], []
                        for tb in range(NTB):
                            xt = xs.tile([128, TB], F16, tag=f"xt{tb}")
                            nc.sync.dma_start(
                                xt[:], x16_in[k * 128:(k + 1) * 128,
                                              tb * TB:(tb + 1) * TB])
                            xpt = xps.tile([128, 2 * TB], E4, tag=f"xpt{tb}")
                            nc.sync.dma_start(
                                xpt[:, 0:TB], xr8_in[k * 128:(k + 1) * 128,
                                                     tb * TB:(tb + 1) * TB])
                            # slot1 moving plane: e4m3(x) cast from the fp16 tile
                            nc.scalar.activation(
                                xpt[:, TB:2 * TB], xt[:],
                                mybir.ActivationFunctionType.Copy, scale=0.125)
                            xts.append(xt)
                            xpts.append(xpt)
                        if fake_dma:
                            fake_tiles = (xts, xpts)
                    for eh in range(2):
                        w16c = w16v[:, k, eh * 128:(eh + 1) * 128]
                        wpc = wpv[:, k, :, eh * 128:(eh + 1) * 128]
                        for tb in range(NTB):
                            nc.tensor.matmul(
                                ps[eh][tb][:], w16c, xts[tb][:],
                                start=(k == 0), stop=False)
                        for tb in range(NTB):
                            nc.tensor.matmul(
                                ps[eh][tb][:], wpc,
                                xpts[tb][:].rearrange("p (two t) -> p two t", two=2),
                                start=False, stop=(k == NK - 1),
                                perf_mode=mybir.MatmulPerfMode.DoubleRow)

                for tb in range(NTB):
                    sig = [sigp.tile([128, TB], F32, tag=f"sig_{eh}",
                                     name=f"sig_{eh}_{tb}") for eh in range(2)]
                    for eh in range(2):
                        nc.scalar.activation(
                            sig[eh][:], ps[eh][tb][:],
                            mybir.ActivationFunctionType.Sigmoid,
                            scale=1.0 / 131072.0)
                    if skip_topk:
                        for eh in range(2):
                            nc.gpsimd.dma_start(
                                wout[(tb * 2 + eh) * 256:(tb * 2 + eh) * 256 + 2, :],
                                sig[eh][0:2, 0:8])
                        continue
                    for col in range(TB // 128):
                        tt = tb * (TB // 128) + col
                        scores = scp.tile([128, E], F32, tag="scores")
                        for eh in range(2):
                            tp = tps.tile([128, 128], F32, tag="tp")
                            nc.tensor.transpose(
                                tp[:], sig[eh][:, col * 128:(col + 1) * 128],
                                ident[:])
                            nc.scalar.copy(scores[:, eh * 128:(eh + 1) * 128], tp[:])
                        _topk_chain(nc, chain, scores, bias_b,
                                    wout[tt * 128:(tt + 1) * 128, :],
                                    iout[tt * 128:(tt + 1) * 128, :])
    nc.compile()
    return nc


def host_prep(x, weight, bias):
    x = np.ascontiguousarray(np.asarray(x, dtype=np.float32))
    weight = np.ascontiguousarray(np.asarray(weight, dtype=np.float32))
    bias = np.asarray(bias, dtype=np.float32)
    e4 = ml_dtypes.float8_e4m3

    x16_all = (x * 8.0).astype(np.float16)                 # [N, K]
    xr_all = x - x16_all.astype(np.float32) * 0.125
    xr8_all = (xr_all * 2048.0).astype(e4)                 # [N, K]

    wT = weight.T                                          # [K, E]
    w16T = (wT * 16384.0).astype(np.float16)
    wr = wT - w16T.astype(np.float32) / 16384.0
    wpT = np.stack([(wT * 64.0), (wr * 131072.0)], axis=1).astype(e4)  # [K,2,E]
    w16T = np.ascontiguousarray(w16T)
    wpT = np.ascontiguousarray(wpT)
    biasb = np.ascontiguousarray(np.broadcast_to(bias, (128, E)))

    in_maps = []
    for c in range(N_CORES):
        sl = slice(c * T_CORE, (c + 1) * T_CORE)
        in_maps.append({
            "x16": np.ascontiguousarray(x16_all[sl].T),
            "xr8": np.ascontiguousarray(xr8_all[sl].T),
            "w16": w16T,
            "wp": wpT,
            "biasb": biasb,
        })
    return in_maps


_CACHED = {}


def kernel(x, token_mask, weight, bias):
    in_maps = host_prep(x, weight, bias)
    if "nc" not in _CACHED:
        _CACHED["nc"] = build_kernel()
    nc = _CACHED["nc"]
    res = run_bass_kernel_spmd(nc, in_maps, core_ids=list(range(N_CORES)))
    weights_full = np.concatenate([r["wout"] for r in res.results], axis=0)
    idx_full = np.concatenate([r["iout"] for r in res.results], axis=0)
    return weights_full.astype(np.float32), idx_full.astype(np.int32)
